# revision 7
# baseline (speedup 1.0000x reference)
"""Trainium2 Bass kernel for nn_BandwidthConstrainedComm.

GNN message passing: per batch element, N=256 agents each generate a
message (MLP -> compress -> decompress), compute pairwise bilinear
relevance scores, pick top-K=8 senders (softmax gated), aggregate their
messages, and run a receiver MLP over [obs, agg].

Sharding: pure data parallel over batch B=128 -> 16 per core x 8 cores.

Implementation notes:
  - transposed [feature, agent] layout everywhere; obs transposed once
    on the host (data staging) so there are no on-chip obs transposes.
  - main-path matmuls in float32r (4x faster than fp32 on the PE at
    free-dim >= 256; ~1.5e-4 rel err measured on HW). The message /
    gate-aggregation path runs in bf16: messages are ~1e-3 of the
    output magnitude, so bf16 there is invisible in the result.
  - top-8 via the DVE Max8 instruction over exp'd scores, gates built
    densely with a fused (E >= t)*E scalar_tensor_tensor whose
    accum_out gives the softmax denominator. Gates are applied as
    [N,N] @ [N,MSG] matmuls -- no gather.
  - softmax without max subtraction (scores bounded ~30 -> exp finite).
  - decompress matmul computes messages directly in [sender, msg]
    layout with the bias folded in via an appended ones-row (k=33).
  - batches processed in pairs so most free dims are 512.
"""

import sys

sys.path.insert(0, "/opt/trn_rl_repo")

import numpy as np

# problem dims (hardcoded per contract)
B, N, D = 128, 256, 256
MSG, CD, K = 64, 32, 8
H1, H2 = 128, 256
NCORES = 8
BPC = B // NCORES  # batches per core

_CACHE = {}


def build_program(bpc=BPC, passes=1):
    import concourse.bacc as bacc
    import concourse.mybir as mybir
    import concourse.tile as tile
    from concourse.masks import make_identity
    from contextlib import ExitStack

    dt = mybir.dt
    f32, f32r, bf16 = dt.float32, dt.float32r, dt.bfloat16
    AF = mybir.ActivationFunctionType
    OP = mybir.AluOpType

    assert bpc % 2 == 0
    npairs = bpc // 2

    nc = bacc.Bacc("TRN2", target_bir_lowering=False, debug=False,
                   num_devices=NCORES)

    obsT_d = nc.dram_tensor("obsT", [bpc, D, N], f32, kind="ExternalInput")
    W1_d = nc.dram_tensor("W1", [D, H1], f32, kind="ExternalInput")
    b1_d = nc.dram_tensor("b1", [H1], f32, kind="ExternalInput")
    W2_d = nc.dram_tensor("W2", [H1, MSG], f32, kind="ExternalInput")
    b2_d = nc.dram_tensor("b2", [MSG], f32, kind="ExternalInput")
    Wc_d = nc.dram_tensor("Wc", [MSG, CD], f32, kind="ExternalInput")
    bc_d = nc.dram_tensor("bc", [CD], f32, kind="ExternalInput")
    Wd_d = nc.dram_tensor("Wd", [CD, MSG], f32, kind="ExternalInput")
    bd_d = nc.dram_tensor("bd", [MSG], f32, kind="ExternalInput")
    Wbil_d = nc.dram_tensor("Wbil", [D, D], f32, kind="ExternalInput")
    Wr1_d = nc.dram_tensor("Wr1", [D + MSG, H2], f32, kind="ExternalInput")
    br1_d = nc.dram_tensor("br1", [H2], f32, kind="ExternalInput")
    Wr2_d = nc.dram_tensor("Wr2", [H2, D], f32, kind="ExternalInput")
    br2_d = nc.dram_tensor("br2", [D], f32, kind="ExternalInput")
    out_d = nc.dram_tensor("out", [bpc, N, D], f32, kind="ExternalOutput")

    with tile.TileContext(nc) as tc, ExitStack() as ctx:
        wp = ctx.enter_context(tc.tile_pool(name="wp", bufs=1))
        dp = ctx.enter_context(tc.tile_pool(name="dp", bufs=2))
        sp = ctx.enter_context(tc.tile_pool(name="sp", bufs=2))
        pp = ctx.enter_context(tc.tile_pool(name="pp", bufs=1, space="PSUM"))

        # PSUM banks (8 x 2KB): mlpmix 2, tmp 2, sg 2, rout 2

        # ---------------- one-time setup ----------------
        ident = wp.tile([128, 128], f32)
        make_identity(nc, ident[:])
        ident_b = wp.tile([128, 128], bf16)
        nc.vector.tensor_copy(ident_b[:], ident[:])
        warm_ps = pp.tile([128, 128], f32, tag="mlpmix", bufs=2)
        nc.tensor.transpose(warm_ps[:], ident[:], ident[:])

        def load_cast(dram_ap, shape, name, cdt):
            t_f = wp.tile(shape, f32, name=name + "_f")
            nc.sync.dma_start(t_f[:], dram_ap)
            t_r = wp.tile(shape, cdt, name=name + "_r")
            nc.vector.tensor_copy(t_r[:], t_f[:])
            return t_r

        W1_r0 = load_cast(W1_d[0:128, :], [128, H1], "W1a", f32r)
        W1_r1 = load_cast(W1_d[128:256, :], [128, H1], "W1b", f32r)
        W2_b = load_cast(W2_d[:], [H1, MSG], "W2", bf16)
        Wc_b = load_cast(Wc_d[:], [MSG, CD], "Wc", bf16)
        Wb_r0 = load_cast(Wbil_d[0:128, :], [128, D], "Wba", f32r)
        Wb_r1 = load_cast(Wbil_d[128:256, :], [128, D], "Wbb", f32r)
        Wr1_r0 = load_cast(Wr1_d[0:128, :], [128, H2], "Wr1a", f32r)
        Wr1_r1 = load_cast(Wr1_d[128:256, :], [128, H2], "Wr1b", f32r)
        Wr1_r2 = load_cast(Wr1_d[256:320, :], [MSG, H2], "Wr1c", f32r)
        Wr2_r0 = load_cast(Wr2_d[0:128, :], [128, D], "Wr2a", f32r)
        Wr2_r1 = load_cast(Wr2_d[128:256, :], [128, D], "Wr2b", f32r)

        # Wd with bd folded in as a 33rd contraction row (bf16)
        Wd_f = wp.tile([CD, MSG], f32)
        nc.sync.dma_start(Wd_f[:], Wd_d[:])
        bd_row_f = wp.tile([1, MSG], f32)
        nc.sync.dma_start(bd_row_f[:],
                          bd_d[:].rearrange("(o m) -> o m", o=1))
        Wd_bd_b = wp.tile([CD + 1, MSG], bf16)
        nc.vector.tensor_copy(Wd_bd_b[0:CD, :], Wd_f[:])
        nc.vector.tensor_copy(Wd_bd_b[CD:CD + 1, :], bd_row_f[:])

        def load_bias(dram, p, name, off=0):
            t = wp.tile([p, 1], f32, name=name)
            nc.sync.dma_start(
                t[:], dram[off:off + p].rearrange("(p o) -> p o", o=1))
            return t

        b1_sb = load_bias(b1_d, H1, "b1s")
        b2_sb = load_bias(b2_d, MSG, "b2s")
        bc_sb = load_bias(bc_d, CD, "bcs")
        br1_sb0 = load_bias(br1_d, 128, "br1s0")
        br1_sb1 = load_bias(br1_d, 128, "br1s1", off=128)

        # ones row + br2 row for folding br2 into the last matmul
        ones_f = wp.tile([1, 128], f32)
        nc.vector.memset(ones_f[:], 1.0)
        ones_r = wp.tile([1, 128], f32r)
        nc.vector.tensor_copy(ones_r[:], ones_f[:])
        br2row_f = wp.tile([1, D], f32)
        nc.sync.dma_start(br2row_f[:],
                          br2_d[:].rearrange("(o d) -> o d", o=1))
        br2row_r = wp.tile([1, D], f32r)
        nc.vector.tensor_copy(br2row_r[:], br2row_f[:])

        # ---------------- main loop over batch pairs ----------------
        for _ in range(passes):
            for p in range(npairs):
                b0 = 2 * p
                od_r = []
                for dc in range(2):
                    of = dp.tile([128, 2, N], f32, name=f"od{dc}_f",
                                 tag=f"od{dc}f")
                    nc.sync.dma_start(
                        of[:], obsT_d[b0:b0 + 2, 128 * dc:128 * (dc + 1), :]
                        .rearrange("b d n -> d b n"))
                    orr = dp.tile([128, 2 * N], f32r, name=f"od{dc}_r",
                                  tag=f"od{dc}r")
                    nc.vector.tensor_copy(
                        orr[:], of[:].rearrange("d b n -> d (b n)"))
                    od_r.append(orr)

                # ---- message MLP (pair-wide, bf16 after first layer) ----
                hT_ps = pp.tile([H1, 2 * N], f32, tag="mlpmix", bufs=2)
                nc.tensor.matmul(hT_ps[:], W1_r0[:], od_r[0][:],
                                 start=True, stop=False)
                nc.tensor.matmul(hT_ps[:], W1_r1[:], od_r[1][:],
                                 start=False, stop=True)
                hT_b = sp.tile([H1, 2 * N], bf16, name="hT_b")
                nc.scalar.activation(hT_b[:], hT_ps[:], AF.Relu,
                                     bias=b1_sb[:])

                mgT_ps = pp.tile([MSG, 2 * N], f32, tag="mlpmix", bufs=2)
                nc.tensor.matmul(mgT_ps[:], W2_b[:], hT_b[:],
                                 start=True, stop=True)
                mgT_b = sp.tile([MSG, 2 * N], bf16, name="mgT_b")
                nc.vector.tensor_scalar_add(mgT_b[:], mgT_ps[:], b2_sb[:])

                cT_ps = pp.tile([CD, 2 * N], f32, tag="mlpmix", bufs=2)
                nc.tensor.matmul(cT_ps[:], Wc_b[:], mgT_b[:],
                                 start=True, stop=True)
                cT_b = sp.tile([CD + 1, 2 * N], bf16, name="cT_b")
                nc.vector.tensor_scalar_add(cT_b[0:CD, :], cT_ps[:],
                                            bc_sb[:])
                nc.vector.memset(cT_b[CD:CD + 1, :], 1.0)

                # ---- bilinear tmp (pair-wide, fp32r) ----
                tmpT_r = []
                for ec in range(2):
                    tps = pp.tile([128, 2 * N], f32, tag="tmp", bufs=2,
                                  name=f"tmp{ec}_ps")
                    nc.tensor.matmul(tps[:],
                                     Wb_r0[:, 128 * ec:128 * (ec + 1)],
                                     od_r[0][:], start=True, stop=False)
                    nc.tensor.matmul(tps[:],
                                     Wb_r1[:, 128 * ec:128 * (ec + 1)],
                                     od_r[1][:], start=False, stop=True)
                    trr = sp.tile([128, 2 * N], f32r, name=f"tmp{ec}_r",
                                  tag=f"tmp{ec}r")
                    nc.scalar.activation(trr[:], tps[:], AF.Copy)
                    tmpT_r.append(trr)

                aggT_ps = pp.tile([MSG, 2 * N], f32, tag="mlpmix", bufs=2)

                for bi in range(2):
                    boff = bi * N

                    # messages directly in [sender, msg] layout, bias
                    # folded via the ones-row (k = CD+1)
                    msn_ps = pp.tile([128, 2, MSG], f32, tag="mlpmix",
                                     bufs=2, name="msn_ps")
                    for jc in range(2):
                        nc.tensor.matmul(
                            msn_ps[:, jc, :],
                            cT_b[:, boff + 128 * jc:boff + 128 * (jc + 1)],
                            Wd_bd_b[:], start=True, stop=True)
                    msgs_b = sp.tile([128, 2, MSG], bf16, name="msgs_b")
                    nc.vector.tensor_copy(msgs_b[:], msn_ps[:])

                    # scores for this batch: [i_chunk(128), (ic, j)(512)]
                    s_ps = pp.tile([128, 2, N], f32, tag="sg", bufs=2,
                                   name="s_ps")
                    for ic in range(2):
                        ioff = boff + 128 * ic
                        nc.tensor.matmul(s_ps[:, ic, :],
                                         tmpT_r[0][:, ioff:ioff + 128],
                                         od_r[0][:, boff:boff + N],
                                         start=True, stop=False)
                        nc.tensor.matmul(s_ps[:, ic, :],
                                         tmpT_r[1][:, ioff:ioff + 128],
                                         od_r[1][:, boff:boff + N],
                                         start=False, stop=True)

                    # gating
                    E = sp.tile([128, 2, N], f32, name="E")
                    nc.scalar.activation(E[:], s_ps[:], AF.Exp)
                    Gt_ps = pp.tile([128, 2, N], bf16, tag="sg", bufs=2,
                                    name="Gt_ps")
                    for ic in range(2):
                        top8 = sp.tile([128, 8], f32, name="top8")
                        nc.vector.max(out=top8[:], in_=E[:, ic, :])
                        U = sp.tile([128, N], f32, name="U")
                        den = sp.tile([128, 1], f32, name="den")
                        nc.vector.scalar_tensor_tensor(
                            out=U[:], in0=E[:, ic, :], scalar=top8[:, 7:8],
                            in1=E[:, ic, :], op0=OP.is_ge, op1=OP.mult,
                            accum_out=den[:])
                        rden = sp.tile([128, 1], f32, name="rden")
                        nc.vector.reciprocal(rden[:], den[:])
                        G_b = sp.tile([128, N], bf16, name="G_b")
                        nc.gpsimd.tensor_scalar_mul(G_b[:], U[:],
                                                    rden[:, 0:1])
                        for jc in range(2):
                            nc.tensor.transpose(
                                Gt_ps[:, jc, 128 * ic:128 * (ic + 1)],
                                G_b[:, 128 * jc:128 * (jc + 1)],
                                ident_b[:])

                    Gt_b = sp.tile([128, 2, N], bf16, name="Gt_b")
                    nc.scalar.activation(Gt_b[:], Gt_ps[:], AF.Copy)

                    # aggT[m, i] = sum_j msgs[j, m] * Gt[j, i]
                    nc.tensor.matmul(aggT_ps[:, boff:boff + N],
                                     msgs_b[:, 0, :], Gt_b[:, 0, :],
                                     start=True, stop=False)
                    nc.tensor.matmul(aggT_ps[:, boff:boff + N],
                                     msgs_b[:, 1, :], Gt_b[:, 1, :],
                                     start=False, stop=True)

                aggT_r = sp.tile([MSG, 2 * N], f32r, name="aggT_r")
                nc.vector.tensor_copy(aggT_r[:], aggT_ps[:])

                # ---- receiver MLP (pair-wide, fp32r) ----
                rT_r = []
                for mi in range(2):
                    rps = pp.tile([128, 2 * N], f32, tag="rout", bufs=2,
                                  name=f"r{mi}_ps")
                    ms = 128 * mi
                    nc.tensor.matmul(rps[:], Wr1_r0[:, ms:ms + 128],
                                     od_r[0][:], start=True, stop=False)
                    nc.tensor.matmul(rps[:], Wr1_r1[:, ms:ms + 128],
                                     od_r[1][:], start=False, stop=False)
                    nc.tensor.matmul(rps[:], Wr1_r2[:, ms:ms + 128],
                                     aggT_r[:], start=False, stop=True)
                    rr = sp.tile([128, 2 * N], f32r, name=f"r{mi}_r",
                                 tag=f"r{mi}r")
                    nc.scalar.activation(
                        rr[:], rps[:], AF.Relu,
                        bias=(br1_sb0 if mi == 0 else br1_sb1)[:])
                    rT_r.append(rr)

                for bi in range(2):
                    b = b0 + bi
                    boff = bi * N
                    out_ps = pp.tile([128, 2, D], f32, tag="mlpmix",
                                     bufs=2, name="out_ps")
                    for ni in range(2):
                        ls = boff + 128 * ni
                        nc.tensor.matmul(out_ps[:, ni, :],
                                         rT_r[0][:, ls:ls + 128],
                                         Wr2_r0[:], start=True, stop=False)
                        nc.tensor.matmul(out_ps[:, ni, :],
                                         rT_r[1][:, ls:ls + 128],
                                         Wr2_r1[:], start=False, stop=False)
                        nc.tensor.matmul(out_ps[:, ni, :],
                                         ones_r[:, 0:128], br2row_r[:],
                                         start=False, stop=True)
                    out_sb = sp.tile([128, 2, D], f32, name="out_sb")
                    nc.scalar.activation(out_sb[:], out_ps[:], AF.Copy)
                    nc.sync.dma_start(
                        out_d[b].rearrange("(c p) d -> p c d", p=128),
                        out_sb[:])

    nc.compile()
    return nc


def _np_inputs_for_core(inputs, core):
    obs = inputs["obs_all"]
    lo = core * BPC
    obsT = np.ascontiguousarray(
        obs[lo:lo + BPC].transpose(0, 2, 1)).astype(np.float32)
    m = {"obsT": obsT}
    for k in ("W1", "b1", "W2", "b2", "Wc", "bc", "Wd", "bd", "Wbil",
              "Wr1", "br1", "Wr2", "br2"):
        m[k] = np.ascontiguousarray(inputs[k]).astype(np.float32)
    return m


def kernel(**inputs):
    from concourse.bass_utils import run_bass_kernel_spmd

    if "prog" not in _CACHE:
        _CACHE["prog"] = build_program(BPC)
    nc = _CACHE["prog"]

    core_ids = list(range(NCORES))
    in_maps = [_np_inputs_for_core(inputs, c) for c in core_ids]
    res = run_bass_kernel_spmd(nc, in_maps, core_ids)
    out = np.concatenate([res.results[c]["out"] for c in core_ids], axis=0)
    return out.astype(np.float32)


# revision 10
# speedup vs baseline: 251.2292x; 251.2292x over previous
"""Trainium2 Bass kernel for nn_BandwidthConstrainedComm.

GNN message passing: per batch element, N=256 agents each generate a
message (MLP -> compress -> decompress), compute pairwise bilinear
relevance scores, pick top-K=8 senders (softmax gated), aggregate their
messages, and run a receiver MLP over [obs, agg].

Sharding: pure data parallel over batch B=128 -> 16 per core x 8 cores.

Implementation notes:
  - transposed [feature, agent] layout everywhere; obs transposed once
    on the host (data staging) so there are no on-chip obs transposes.
  - main-path matmuls in float32r (4x faster than fp32 on the PE at
    free-dim >= 256; ~1.5e-4 rel err measured on HW). The message /
    gate-aggregation path runs in bf16: messages are ~1e-3 of the
    output magnitude, so bf16 there is invisible in the result.
  - top-8 via the DVE Max8 instruction over exp'd scores, gates built
    densely with a fused (E >= t)*E scalar_tensor_tensor whose
    accum_out gives the softmax denominator. Gates are applied as
    [N,N] @ [N,MSG] matmuls -- no gather.
  - softmax without max subtraction (scores bounded ~30 -> exp finite).
  - decompress matmul computes messages directly in [sender, msg]
    layout with the bias folded in via an appended ones-row (k=33).
  - batches processed in pairs so most free dims are 512.
"""

import sys

sys.path.insert(0, "/opt/trn_rl_repo")

import numpy as np

# problem dims (hardcoded per contract)
B, N, D = 128, 256, 256
MSG, CD, K = 64, 32, 8
H1, H2 = 128, 256
NCORES = 8
BPC = B // NCORES  # batches per core

_CACHE = {}


def build_program(bpc=BPC, passes=1):
    import concourse.bacc as bacc
    import concourse.mybir as mybir
    import concourse.tile as tile
    from concourse.masks import make_identity
    from contextlib import ExitStack

    dt = mybir.dt
    f32, f32r, bf16 = dt.float32, dt.float32r, dt.bfloat16
    AF = mybir.ActivationFunctionType
    OP = mybir.AluOpType

    assert bpc % 2 == 0
    npairs = bpc // 2

    nc = bacc.Bacc("TRN2", target_bir_lowering=False, debug=False,
                   num_devices=NCORES)

    obsT_d = nc.dram_tensor("obsT", [bpc, D, N], f32, kind="ExternalInput")
    W1_d = nc.dram_tensor("W1", [D, H1], f32, kind="ExternalInput")
    b1_d = nc.dram_tensor("b1", [H1], f32, kind="ExternalInput")
    W2_d = nc.dram_tensor("W2", [H1, MSG], f32, kind="ExternalInput")
    b2_d = nc.dram_tensor("b2", [MSG], f32, kind="ExternalInput")
    Wc_d = nc.dram_tensor("Wc", [MSG, CD], f32, kind="ExternalInput")
    bc_d = nc.dram_tensor("bc", [CD], f32, kind="ExternalInput")
    Wd_d = nc.dram_tensor("Wd", [CD, MSG], f32, kind="ExternalInput")
    bd_d = nc.dram_tensor("bd", [MSG], f32, kind="ExternalInput")
    Wbil_d = nc.dram_tensor("Wbil", [D, D], f32, kind="ExternalInput")
    Wr1_d = nc.dram_tensor("Wr1", [D + MSG, H2], f32, kind="ExternalInput")
    br1_d = nc.dram_tensor("br1", [H2], f32, kind="ExternalInput")
    Wr2_d = nc.dram_tensor("Wr2", [H2, D], f32, kind="ExternalInput")
    br2_d = nc.dram_tensor("br2", [D], f32, kind="ExternalInput")
    out_d = nc.dram_tensor("out", [bpc, N, D], f32, kind="ExternalOutput")

    with tile.TileContext(nc) as tc, ExitStack() as ctx:
        wp = ctx.enter_context(tc.tile_pool(name="wp", bufs=1))
        dp = ctx.enter_context(tc.tile_pool(name="dp", bufs=2))
        sp = ctx.enter_context(tc.tile_pool(name="sp", bufs=2))
        pp = ctx.enter_context(tc.tile_pool(name="pp", bufs=1, space="PSUM"))

        # PSUM banks (8 x 2KB): mlpmix 2, tmp 2, sg 2, rout 2

        # ---------------- one-time setup ----------------
        ident = wp.tile([128, 128], f32)
        make_identity(nc, ident[:])
        ident_b = wp.tile([128, 128], bf16)
        nc.vector.tensor_copy(ident_b[:], ident[:])
        warm_ps = pp.tile([128, 128], f32, tag="mlpmix", bufs=2)
        nc.tensor.transpose(warm_ps[:], ident[:], ident[:])

        def load_cast(dram_ap, shape, name, cdt):
            t_f = wp.tile(shape, f32, name=name + "_f")
            nc.sync.dma_start(t_f[:], dram_ap)
            t_r = wp.tile(shape, cdt, name=name + "_r")
            nc.vector.tensor_copy(t_r[:], t_f[:])
            return t_r

        W1_r0 = load_cast(W1_d[0:128, :], [128, H1], "W1a", f32r)
        W1_r1 = load_cast(W1_d[128:256, :], [128, H1], "W1b", f32r)
        W2_b = load_cast(W2_d[:], [H1, MSG], "W2", bf16)
        Wc_b = load_cast(Wc_d[:], [MSG, CD], "Wc", bf16)
        Wb_r0 = load_cast(Wbil_d[0:128, :], [128, D], "Wba", f32r)
        Wb_r1 = load_cast(Wbil_d[128:256, :], [128, D], "Wbb", f32r)
        Wr1_r0 = load_cast(Wr1_d[0:128, :], [128, H2], "Wr1a", f32r)
        Wr1_r1 = load_cast(Wr1_d[128:256, :], [128, H2], "Wr1b", f32r)
        Wr1_r2 = load_cast(Wr1_d[256:320, :], [MSG, H2], "Wr1c", f32r)
        Wr2_r0 = load_cast(Wr2_d[0:128, :], [128, D], "Wr2a", f32r)
        Wr2_r1 = load_cast(Wr2_d[128:256, :], [128, D], "Wr2b", f32r)

        # Wd with bd folded in as a 33rd contraction row (bf16)
        Wd_f = wp.tile([CD, MSG], f32)
        nc.sync.dma_start(Wd_f[:], Wd_d[:])
        bd_row_f = wp.tile([1, MSG], f32)
        nc.sync.dma_start(bd_row_f[:],
                          bd_d[:].rearrange("(o m) -> o m", o=1))
        Wd_bd_b = wp.tile([CD + 1, MSG], bf16)
        nc.vector.tensor_copy(Wd_bd_b[0:CD, :], Wd_f[:])
        nc.vector.tensor_copy(Wd_bd_b[CD:CD + 1, :], bd_row_f[:])

        def load_bias(dram, p, name, off=0):
            t = wp.tile([p, 1], f32, name=name)
            nc.sync.dma_start(
                t[:], dram[off:off + p].rearrange("(p o) -> p o", o=1))
            return t

        b1_sb = load_bias(b1_d, H1, "b1s")
        b2_sb = load_bias(b2_d, MSG, "b2s")
        bc_sb = load_bias(bc_d, CD, "bcs")
        br1_sb0 = load_bias(br1_d, 128, "br1s0")
        br1_sb1 = load_bias(br1_d, 128, "br1s1", off=128)

        # ones row + br2 row for folding br2 into the last matmul
        ones_f = wp.tile([1, 128], f32)
        nc.vector.memset(ones_f[:], 1.0)
        ones_r = wp.tile([1, 128], f32r)
        nc.vector.tensor_copy(ones_r[:], ones_f[:])
        br2row_f = wp.tile([1, D], f32)
        nc.sync.dma_start(br2row_f[:],
                          br2_d[:].rearrange("(o d) -> o d", o=1))
        br2row_r = wp.tile([1, D], f32r)
        nc.vector.tensor_copy(br2row_r[:], br2row_f[:])

        # persistent double-buffered compressed-message tiles with a
        # constant ones-row (row CD) for the folded bd bias
        cT_tiles = []
        for i in range(2):
            t = wp.tile([CD + 1, 2 * N], bf16, name=f"cTp{i}")
            nc.vector.memset(t[CD:CD + 1, :], 1.0)
            cT_tiles.append(t)

        # ---------------- main loop over batch pairs ----------------
        for _ in range(passes):
            for p in range(npairs):
                b0 = 2 * p
                od_r = []
                for dc in range(2):
                    of = dp.tile([128, 2, N], f32, name=f"od{dc}_f",
                                 tag=f"od{dc}f")
                    nc.sync.dma_start(
                        of[:], obsT_d[b0:b0 + 2, 128 * dc:128 * (dc + 1), :]
                        .rearrange("b d n -> d b n"))
                    orr = dp.tile([128, 2 * N], f32r, name=f"od{dc}_r",
                                  tag=f"od{dc}r")
                    nc.vector.tensor_copy(
                        orr[:], of[:].rearrange("d b n -> d (b n)"))
                    od_r.append(orr)

                # ---- message MLP (pair-wide, bf16 after first layer) ----
                hT_ps = pp.tile([H1, 2 * N], f32, tag="mlpmix", bufs=2)
                nc.tensor.matmul(hT_ps[:], W1_r0[:], od_r[0][:],
                                 start=True, stop=False)
                nc.tensor.matmul(hT_ps[:], W1_r1[:], od_r[1][:],
                                 start=False, stop=True)
                hT_b = sp.tile([H1, 2 * N], bf16, name="hT_b")
                nc.scalar.activation(hT_b[:], hT_ps[:], AF.Relu,
                                     bias=b1_sb[:])

                mgT_ps = pp.tile([MSG, 2 * N], f32, tag="mlpmix", bufs=2)
                nc.tensor.matmul(mgT_ps[:], W2_b[:], hT_b[:],
                                 start=True, stop=True)
                mgT_b = sp.tile([MSG, 2 * N], bf16, name="mgT_b")
                nc.vector.tensor_scalar_add(mgT_b[:], mgT_ps[:], b2_sb[:])

                cT_ps = pp.tile([CD, 2 * N], f32, tag="mlpmix", bufs=2)
                nc.tensor.matmul(cT_ps[:], Wc_b[:], mgT_b[:],
                                 start=True, stop=True)
                cT_b = cT_tiles[p % 2]
                nc.vector.tensor_scalar_add(cT_b[0:CD, :], cT_ps[:],
                                            bc_sb[:])

                # ---- bilinear tmp (pair-wide, fp32r) ----
                tmpT_r = []
                for ec in range(2):
                    tps = pp.tile([128, 2 * N], f32, tag="tmp", bufs=2,
                                  name=f"tmp{ec}_ps")
                    nc.tensor.matmul(tps[:],
                                     Wb_r0[:, 128 * ec:128 * (ec + 1)],
                                     od_r[0][:], start=True, stop=False)
                    nc.tensor.matmul(tps[:],
                                     Wb_r1[:, 128 * ec:128 * (ec + 1)],
                                     od_r[1][:], start=False, stop=True)
                    trr = sp.tile([128, 2 * N], f32r, name=f"tmp{ec}_r",
                                  tag=f"tmp{ec}r")
                    nc.scalar.activation(trr[:], tps[:], AF.Copy)
                    tmpT_r.append(trr)

                aggT_ps = pp.tile([MSG, 2 * N], f32, tag="mlpmix", bufs=2)

                for bi in range(2):
                    boff = bi * N

                    # messages directly in [sender, msg] layout, bias
                    # folded via the ones-row (k = CD+1)
                    msn_ps = pp.tile([128, 2, MSG], f32, tag="mlpmix",
                                     bufs=2, name="msn_ps")
                    for jc in range(2):
                        nc.tensor.matmul(
                            msn_ps[:, jc, :],
                            cT_b[:, boff + 128 * jc:boff + 128 * (jc + 1)],
                            Wd_bd_b[:], start=True, stop=True)
                    msgs_b = sp.tile([128, 2, MSG], bf16, name="msgs_b")
                    nc.vector.tensor_copy(msgs_b[:], msn_ps[:])

                    # scores for this batch: [i_chunk(128), (ic, j)(512)]
                    s_ps = pp.tile([128, 2, N], f32, tag="sg", bufs=2,
                                   name="s_ps")
                    for ic in range(2):
                        ioff = boff + 128 * ic
                        nc.tensor.matmul(s_ps[:, ic, :],
                                         tmpT_r[0][:, ioff:ioff + 128],
                                         od_r[0][:, boff:boff + N],
                                         start=True, stop=False)
                        nc.tensor.matmul(s_ps[:, ic, :],
                                         tmpT_r[1][:, ioff:ioff + 128],
                                         od_r[1][:, boff:boff + N],
                                         start=False, stop=True)

                    # gating (bf16: message path is ~1e-3 of the output,
                    # so bf16 gate errors are invisible)
                    E = sp.tile([128, 2, N], bf16, name="E")
                    nc.scalar.activation(E[:], s_ps[:], AF.Exp)
                    Gt_ps = pp.tile([128, 2, N], bf16, tag="sg", bufs=2,
                                    name="Gt_ps")
                    for ic in range(2):
                        top8 = sp.tile([128, 8], bf16, name="top8")
                        nc.vector.max(out=top8[:], in_=E[:, ic, :])
                        U = sp.tile([128, N], bf16, name="U")
                        den = sp.tile([128, 1], f32, name="den")
                        nc.vector.scalar_tensor_tensor(
                            out=U[:], in0=E[:, ic, :], scalar=top8[:, 7:8],
                            in1=E[:, ic, :], op0=OP.is_ge, op1=OP.mult,
                            accum_out=den[:])
                        rden = sp.tile([128, 1], f32, name="rden")
                        nc.vector.reciprocal(rden[:], den[:])
                        G_b = sp.tile([128, N], bf16, name="G_b")
                        nc.gpsimd.tensor_scalar_mul(G_b[:], U[:],
                                                    rden[:, 0:1])
                        for jc in range(2):
                            nc.tensor.transpose(
                                Gt_ps[:, jc, 128 * ic:128 * (ic + 1)],
                                G_b[:, 128 * jc:128 * (jc + 1)],
                                ident_b[:])

                    Gt_b = sp.tile([128, 2, N], bf16, name="Gt_b")
                    nc.scalar.activation(Gt_b[:], Gt_ps[:], AF.Copy)

                    # aggT[m, i] = sum_j msgs[j, m] * Gt[j, i]
                    nc.tensor.matmul(aggT_ps[:, boff:boff + N],
                                     msgs_b[:, 0, :], Gt_b[:, 0, :],
                                     start=True, stop=False)
                    nc.tensor.matmul(aggT_ps[:, boff:boff + N],
                                     msgs_b[:, 1, :], Gt_b[:, 1, :],
                                     start=False, stop=True)

                aggT_r = sp.tile([MSG, 2 * N], f32r, name="aggT_r")
                nc.vector.tensor_copy(aggT_r[:], aggT_ps[:])

                # ---- receiver MLP (pair-wide, fp32r) ----
                rT_r = []
                for mi in range(2):
                    rps = pp.tile([128, 2 * N], f32, tag="rout", bufs=2,
                                  name=f"r{mi}_ps")
                    ms = 128 * mi
                    nc.tensor.matmul(rps[:], Wr1_r0[:, ms:ms + 128],
                                     od_r[0][:], start=True, stop=False)
                    nc.tensor.matmul(rps[:], Wr1_r1[:, ms:ms + 128],
                                     od_r[1][:], start=False, stop=False)
                    nc.tensor.matmul(rps[:], Wr1_r2[:, ms:ms + 128],
                                     aggT_r[:], start=False, stop=True)
                    rr = sp.tile([128, 2 * N], f32r, name=f"r{mi}_r",
                                 tag=f"r{mi}r")
                    nc.scalar.activation(
                        rr[:], rps[:], AF.Relu,
                        bias=(br1_sb0 if mi == 0 else br1_sb1)[:])
                    rT_r.append(rr)

                for bi in range(2):
                    b = b0 + bi
                    boff = bi * N
                    out_ps = pp.tile([128, 2, D], f32, tag="mlpmix",
                                     bufs=2, name="out_ps")
                    for ni in range(2):
                        ls = boff + 128 * ni
                        nc.tensor.matmul(out_ps[:, ni, :],
                                         rT_r[0][:, ls:ls + 128],
                                         Wr2_r0[:], start=True, stop=False)
                        nc.tensor.matmul(out_ps[:, ni, :],
                                         rT_r[1][:, ls:ls + 128],
                                         Wr2_r1[:], start=False, stop=False)
                        nc.tensor.matmul(out_ps[:, ni, :],
                                         ones_r[:, 0:128], br2row_r[:],
                                         start=False, stop=True)
                    out_sb = sp.tile([128, 2, D], f32, name="out_sb")
                    nc.scalar.activation(out_sb[:], out_ps[:], AF.Copy)
                    nc.sync.dma_start(
                        out_d[b].rearrange("(c p) d -> p c d", p=128),
                        out_sb[:])

    nc.compile()
    return nc


def _np_inputs_for_core(inputs, core):
    obs = inputs["obs_all"]
    lo = core * BPC
    obsT = np.ascontiguousarray(
        obs[lo:lo + BPC].transpose(0, 2, 1)).astype(np.float32)
    m = {"obsT": obsT}
    for k in ("W1", "b1", "W2", "b2", "Wc", "bc", "Wd", "bd", "Wbil",
              "Wr1", "br1", "Wr2", "br2"):
        m[k] = np.ascontiguousarray(inputs[k]).astype(np.float32)
    return m


def kernel(**inputs):
    from concourse.bass_utils import run_bass_kernel_spmd

    if "prog" not in _CACHE:
        _CACHE["prog"] = build_program(BPC)
    nc = _CACHE["prog"]

    core_ids = list(range(NCORES))
    in_maps = [_np_inputs_for_core(inputs, c) for c in core_ids]
    res = run_bass_kernel_spmd(nc, in_maps, core_ids)
    out = np.concatenate([res.results[c]["out"] for c in core_ids], axis=0)
    return out.astype(np.float32)


# revision 11
# speedup vs baseline: 305.9039x; 1.2176x over previous
"""Trainium2 Bass kernel for nn_BandwidthConstrainedComm.

GNN message passing: per batch element, N=256 agents each generate a
message (MLP -> compress -> decompress), compute pairwise bilinear
relevance scores, pick top-K=8 senders (softmax gated), aggregate their
messages, and run a receiver MLP over [obs, agg].

Sharding: pure data parallel over batch B=128 -> 16 per core x 8 cores.

Implementation notes:
  - transposed [feature, agent] layout everywhere; obs transposed once
    on the host (data staging) so there are no on-chip obs transposes.
  - main-path matmuls in float32r (4x faster than fp32 on the PE at
    free-dim >= 256; ~1.5e-4 rel err measured on HW). The message /
    gate-aggregation path runs in bf16: messages are ~1e-3 of the
    output magnitude, so bf16 there is invisible in the result.
  - top-8 via the DVE Max8 instruction over exp'd scores, gates built
    densely with a fused (E >= t)*E scalar_tensor_tensor whose
    accum_out gives the softmax denominator. Gates are applied as
    [N,N] @ [N,MSG] matmuls -- no gather.
  - softmax without max subtraction (scores bounded ~30 -> exp finite).
  - decompress matmul computes messages directly in [sender, msg]
    layout with the bias folded in via an appended ones-row (k=33).
  - batches processed in pairs so most free dims are 512.
"""

import sys

sys.path.insert(0, "/opt/trn_rl_repo")

import numpy as np

# problem dims (hardcoded per contract)
B, N, D = 128, 256, 256
MSG, CD, K = 64, 32, 8
H1, H2 = 128, 256
NCORES = 8
BPC = B // NCORES  # batches per core

_CACHE = {}


def build_program(bpc=BPC, passes=1):
    import concourse.bacc as bacc
    import concourse.mybir as mybir
    import concourse.tile as tile
    from concourse.masks import make_identity
    from contextlib import ExitStack

    dt = mybir.dt
    f32, f32r, bf16 = dt.float32, dt.float32r, dt.bfloat16
    AF = mybir.ActivationFunctionType
    OP = mybir.AluOpType

    assert bpc % 2 == 0
    npairs = bpc // 2

    nc = bacc.Bacc("TRN2", target_bir_lowering=False, debug=False,
                   num_devices=NCORES)

    obsT_d = nc.dram_tensor("obsT", [bpc, D, N], f32, kind="ExternalInput")
    W1_d = nc.dram_tensor("W1", [D, H1], f32, kind="ExternalInput")
    b1_d = nc.dram_tensor("b1", [H1], f32, kind="ExternalInput")
    W2_d = nc.dram_tensor("W2", [H1, MSG], f32, kind="ExternalInput")
    b2_d = nc.dram_tensor("b2", [MSG], f32, kind="ExternalInput")
    Wc_d = nc.dram_tensor("Wc", [MSG, CD], f32, kind="ExternalInput")
    bc_d = nc.dram_tensor("bc", [CD], f32, kind="ExternalInput")
    Wd_d = nc.dram_tensor("Wd", [CD, MSG], f32, kind="ExternalInput")
    bd_d = nc.dram_tensor("bd", [MSG], f32, kind="ExternalInput")
    Wbil_d = nc.dram_tensor("Wbil", [D, D], f32, kind="ExternalInput")
    Wr1_d = nc.dram_tensor("Wr1", [D + MSG, H2], f32, kind="ExternalInput")
    br1_d = nc.dram_tensor("br1", [H2], f32, kind="ExternalInput")
    Wr2_d = nc.dram_tensor("Wr2", [H2, D], f32, kind="ExternalInput")
    br2_d = nc.dram_tensor("br2", [D], f32, kind="ExternalInput")
    out_d = nc.dram_tensor("out", [bpc, N, D], f32, kind="ExternalOutput")

    with tile.TileContext(nc) as tc, ExitStack() as ctx:
        wp = ctx.enter_context(tc.tile_pool(name="wp", bufs=1))
        dp = ctx.enter_context(tc.tile_pool(name="dp", bufs=2))
        sp = ctx.enter_context(tc.tile_pool(name="sp", bufs=2))
        pp = ctx.enter_context(tc.tile_pool(name="pp", bufs=1, space="PSUM"))

        # PSUM banks (8 x 2KB): mlpmix 2, tmp 2, sg 2, rout 2

        # ---------------- one-time setup ----------------
        ident = wp.tile([128, 128], f32)
        make_identity(nc, ident[:])
        ident_b = wp.tile([128, 128], bf16)
        nc.vector.tensor_copy(ident_b[:], ident[:])
        warm_ps = pp.tile([128, 128], f32, tag="mlpmix", bufs=2)
        nc.tensor.transpose(warm_ps[:], ident[:], ident[:])

        def load_cast(dram_ap, shape, name, cdt):
            t_f = wp.tile(shape, f32, name=name + "_f")
            nc.sync.dma_start(t_f[:], dram_ap)
            t_r = wp.tile(shape, cdt, name=name + "_r")
            nc.vector.tensor_copy(t_r[:], t_f[:])
            return t_r

        W1_r0 = load_cast(W1_d[0:128, :], [128, H1], "W1a", bf16)
        W1_r1 = load_cast(W1_d[128:256, :], [128, H1], "W1b", bf16)
        W2_b = load_cast(W2_d[:], [H1, MSG], "W2", bf16)
        Wc_b = load_cast(Wc_d[:], [MSG, CD], "Wc", bf16)
        Wb_r0 = load_cast(Wbil_d[0:128, :], [128, D], "Wba", bf16)
        Wb_r1 = load_cast(Wbil_d[128:256, :], [128, D], "Wbb", bf16)
        Wr1_r0 = load_cast(Wr1_d[0:128, :], [128, H2], "Wr1a", f32r)
        Wr1_r1 = load_cast(Wr1_d[128:256, :], [128, H2], "Wr1b", f32r)
        Wr1_r2 = load_cast(Wr1_d[256:320, :], [MSG, H2], "Wr1c", f32r)
        Wr2_r0 = load_cast(Wr2_d[0:128, :], [128, D], "Wr2a", f32r)
        Wr2_r1 = load_cast(Wr2_d[128:256, :], [128, D], "Wr2b", f32r)

        # Wd with bd folded in as a 33rd contraction row (bf16)
        Wd_f = wp.tile([CD, MSG], f32)
        nc.sync.dma_start(Wd_f[:], Wd_d[:])
        bd_row_f = wp.tile([1, MSG], f32)
        nc.sync.dma_start(bd_row_f[:],
                          bd_d[:].rearrange("(o m) -> o m", o=1))
        Wd_bd_b = wp.tile([CD + 1, MSG], bf16)
        nc.vector.tensor_copy(Wd_bd_b[0:CD, :], Wd_f[:])
        nc.vector.tensor_copy(Wd_bd_b[CD:CD + 1, :], bd_row_f[:])

        def load_bias(dram, p, name, off=0):
            t = wp.tile([p, 1], f32, name=name)
            nc.sync.dma_start(
                t[:], dram[off:off + p].rearrange("(p o) -> p o", o=1))
            return t

        b1_sb = load_bias(b1_d, H1, "b1s")
        b2_sb = load_bias(b2_d, MSG, "b2s")
        bc_sb = load_bias(bc_d, CD, "bcs")
        br1_sb0 = load_bias(br1_d, 128, "br1s0")
        br1_sb1 = load_bias(br1_d, 128, "br1s1", off=128)

        # ones row + br2 row for folding br2 into the last matmul
        ones_f = wp.tile([1, 128], f32)
        nc.vector.memset(ones_f[:], 1.0)
        ones_r = wp.tile([1, 128], f32r)
        nc.vector.tensor_copy(ones_r[:], ones_f[:])
        br2row_f = wp.tile([1, D], f32)
        nc.sync.dma_start(br2row_f[:],
                          br2_d[:].rearrange("(o d) -> o d", o=1))
        br2row_r = wp.tile([1, D], f32r)
        nc.vector.tensor_copy(br2row_r[:], br2row_f[:])

        # persistent double-buffered compressed-message tiles with a
        # constant ones-row (row CD) for the folded bd bias
        cT_tiles = []
        for i in range(2):
            t = wp.tile([CD + 1, 2 * N], bf16, name=f"cTp{i}")
            nc.vector.memset(t[CD:CD + 1, :], 1.0)
            cT_tiles.append(t)

        # ---------------- main loop over batch pairs ----------------
        for _ in range(passes):
            for p in range(npairs):
                b0 = 2 * p
                od_r = []
                od_b = []
                for dc in range(2):
                    of = dp.tile([128, 2, N], f32, name=f"od{dc}_f",
                                 tag=f"od{dc}f")
                    nc.sync.dma_start(
                        of[:], obsT_d[b0:b0 + 2, 128 * dc:128 * (dc + 1), :]
                        .rearrange("b d n -> d b n"))
                    orr = dp.tile([128, 2 * N], f32r, name=f"od{dc}_r",
                                  tag=f"od{dc}r")
                    nc.vector.tensor_copy(
                        orr[:], of[:].rearrange("d b n -> d (b n)"))
                    od_r.append(orr)
                    ob = dp.tile([128, 2 * N], bf16, name=f"od{dc}_b",
                                 tag=f"od{dc}b")
                    nc.vector.tensor_copy(
                        ob[:], of[:].rearrange("d b n -> d (b n)"))
                    od_b.append(ob)

                # ---- message MLP (pair-wide, bf16 after first layer) ----
                hT_ps = pp.tile([H1, 2 * N], f32, tag="mlpmix", bufs=2)
                nc.tensor.matmul(hT_ps[:], W1_r0[:], od_b[0][:],
                                 start=True, stop=False)
                nc.tensor.matmul(hT_ps[:], W1_r1[:], od_b[1][:],
                                 start=False, stop=True)
                hT_b = sp.tile([H1, 2 * N], bf16, name="hT_b")
                nc.scalar.activation(hT_b[:], hT_ps[:], AF.Relu,
                                     bias=b1_sb[:])

                mgT_ps = pp.tile([MSG, 2 * N], f32, tag="mlpmix", bufs=2)
                nc.tensor.matmul(mgT_ps[:], W2_b[:], hT_b[:],
                                 start=True, stop=True)
                mgT_b = sp.tile([MSG, 2 * N], bf16, name="mgT_b")
                nc.vector.tensor_scalar_add(mgT_b[:], mgT_ps[:], b2_sb[:])

                cT_ps = pp.tile([CD, 2 * N], f32, tag="mlpmix", bufs=2)
                nc.tensor.matmul(cT_ps[:], Wc_b[:], mgT_b[:],
                                 start=True, stop=True)
                cT_b = cT_tiles[p % 2]
                nc.vector.tensor_scalar_add(cT_b[0:CD, :], cT_ps[:],
                                            bc_sb[:])

                # ---- bilinear tmp (pair-wide, fp32r) ----
                tmpT_r = []
                for ec in range(2):
                    tps = pp.tile([128, 2 * N], f32, tag="tmp", bufs=2,
                                  name=f"tmp{ec}_ps")
                    nc.tensor.matmul(tps[:],
                                     Wb_r0[:, 128 * ec:128 * (ec + 1)],
                                     od_b[0][:], start=True, stop=False)
                    nc.tensor.matmul(tps[:],
                                     Wb_r1[:, 128 * ec:128 * (ec + 1)],
                                     od_b[1][:], start=False, stop=True)
                    trr = sp.tile([128, 2 * N], bf16, name=f"tmp{ec}_r",
                                  tag=f"tmp{ec}r")
                    nc.scalar.activation(trr[:], tps[:], AF.Copy)
                    tmpT_r.append(trr)

                aggT_ps = pp.tile([MSG, 2 * N], f32, tag="mlpmix", bufs=2)

                for bi in range(2):
                    boff = bi * N

                    # messages directly in [sender, msg] layout, bias
                    # folded via the ones-row (k = CD+1)
                    msn_ps = pp.tile([128, 2, MSG], f32, tag="mlpmix",
                                     bufs=2, name="msn_ps")
                    for jc in range(2):
                        nc.tensor.matmul(
                            msn_ps[:, jc, :],
                            cT_b[:, boff + 128 * jc:boff + 128 * (jc + 1)],
                            Wd_bd_b[:], start=True, stop=True)
                    msgs_b = sp.tile([128, 2, MSG], bf16, name="msgs_b")
                    nc.vector.tensor_copy(msgs_b[:], msn_ps[:])

                    # scores for this batch: [i_chunk(128), (ic, j)(512)]
                    s_ps = pp.tile([128, 2, N], f32, tag="sg", bufs=2,
                                   name="s_ps")
                    for ic in range(2):
                        ioff = boff + 128 * ic
                        nc.tensor.matmul(s_ps[:, ic, :],
                                         tmpT_r[0][:, ioff:ioff + 128],
                                         od_b[0][:, boff:boff + N],
                                         start=True, stop=False)
                        nc.tensor.matmul(s_ps[:, ic, :],
                                         tmpT_r[1][:, ioff:ioff + 128],
                                         od_b[1][:, boff:boff + N],
                                         start=False, stop=True)

                    # gating (bf16: message path is ~1e-3 of the output,
                    # so bf16 gate errors are invisible)
                    E = sp.tile([128, 2, N], bf16, name="E")
                    nc.scalar.activation(E[:], s_ps[:], AF.Exp)
                    Gt_ps = pp.tile([128, 2, N], bf16, tag="sg", bufs=2,
                                    name="Gt_ps")
                    for ic in range(2):
                        top8 = sp.tile([128, 8], bf16, name="top8")
                        nc.vector.max(out=top8[:], in_=E[:, ic, :])
                        U = sp.tile([128, N], bf16, name="U")
                        den = sp.tile([128, 1], f32, name="den")
                        nc.vector.scalar_tensor_tensor(
                            out=U[:], in0=E[:, ic, :], scalar=top8[:, 7:8],
                            in1=E[:, ic, :], op0=OP.is_ge, op1=OP.mult,
                            accum_out=den[:])
                        rden = sp.tile([128, 1], f32, name="rden")
                        nc.vector.reciprocal(rden[:], den[:])
                        G_b = sp.tile([128, N], bf16, name="G_b")
                        nc.vector.tensor_scalar_mul(G_b[:], U[:],
                                                    rden[:, 0:1])
                        for jc in range(2):
                            nc.tensor.transpose(
                                Gt_ps[:, jc, 128 * ic:128 * (ic + 1)],
                                G_b[:, 128 * jc:128 * (jc + 1)],
                                ident_b[:])

                    Gt_b = sp.tile([128, 2, N], bf16, name="Gt_b")
                    nc.scalar.activation(Gt_b[:], Gt_ps[:], AF.Copy)

                    # aggT[m, i] = sum_j msgs[j, m] * Gt[j, i]
                    nc.tensor.matmul(aggT_ps[:, boff:boff + N],
                                     msgs_b[:, 0, :], Gt_b[:, 0, :],
                                     start=True, stop=False)
                    nc.tensor.matmul(aggT_ps[:, boff:boff + N],
                                     msgs_b[:, 1, :], Gt_b[:, 1, :],
                                     start=False, stop=True)

                aggT_r = sp.tile([MSG, 2 * N], f32r, name="aggT_r")
                nc.vector.tensor_copy(aggT_r[:], aggT_ps[:])

                # ---- receiver MLP (pair-wide, fp32r) ----
                rT_r = []
                for mi in range(2):
                    rps = pp.tile([128, 2 * N], f32, tag="rout", bufs=2,
                                  name=f"r{mi}_ps")
                    ms = 128 * mi
                    nc.tensor.matmul(rps[:], Wr1_r0[:, ms:ms + 128],
                                     od_r[0][:], start=True, stop=False)
                    nc.tensor.matmul(rps[:], Wr1_r1[:, ms:ms + 128],
                                     od_r[1][:], start=False, stop=False)
                    nc.tensor.matmul(rps[:], Wr1_r2[:, ms:ms + 128],
                                     aggT_r[:], start=False, stop=True)
                    rr = sp.tile([128, 2 * N], f32r, name=f"r{mi}_r",
                                 tag=f"r{mi}r")
                    nc.scalar.activation(
                        rr[:], rps[:], AF.Relu,
                        bias=(br1_sb0 if mi == 0 else br1_sb1)[:])
                    rT_r.append(rr)

                for bi in range(2):
                    b = b0 + bi
                    boff = bi * N
                    out_ps = pp.tile([128, 2, D], f32, tag="mlpmix",
                                     bufs=2, name="out_ps")
                    for ni in range(2):
                        ls = boff + 128 * ni
                        nc.tensor.matmul(out_ps[:, ni, :],
                                         rT_r[0][:, ls:ls + 128],
                                         Wr2_r0[:], start=True, stop=False)
                        nc.tensor.matmul(out_ps[:, ni, :],
                                         rT_r[1][:, ls:ls + 128],
                                         Wr2_r1[:], start=False, stop=False)
                        nc.tensor.matmul(out_ps[:, ni, :],
                                         ones_r[:, 0:128], br2row_r[:],
                                         start=False, stop=True)
                    out_sb = sp.tile([128, 2, D], f32, name="out_sb")
                    nc.scalar.activation(out_sb[:], out_ps[:], AF.Copy)
                    nc.sync.dma_start(
                        out_d[b].rearrange("(c p) d -> p c d", p=128),
                        out_sb[:])

    nc.compile()
    return nc


def _np_inputs_for_core(inputs, core):
    obs = inputs["obs_all"]
    lo = core * BPC
    obsT = np.ascontiguousarray(
        obs[lo:lo + BPC].transpose(0, 2, 1)).astype(np.float32)
    m = {"obsT": obsT}
    for k in ("W1", "b1", "W2", "b2", "Wc", "bc", "Wd", "bd", "Wbil",
              "Wr1", "br1", "Wr2", "br2"):
        m[k] = np.ascontiguousarray(inputs[k]).astype(np.float32)
    return m


def kernel(**inputs):
    from concourse.bass_utils import run_bass_kernel_spmd

    if "prog" not in _CACHE:
        _CACHE["prog"] = build_program(BPC)
    nc = _CACHE["prog"]

    core_ids = list(range(NCORES))
    in_maps = [_np_inputs_for_core(inputs, c) for c in core_ids]
    res = run_bass_kernel_spmd(nc, in_maps, core_ids)
    out = np.concatenate([res.results[c]["out"] for c in core_ids], axis=0)
    return out.astype(np.float32)


# revision 13
# speedup vs baseline: 376.4655x; 1.2307x over previous
"""Trainium2 Bass kernel for nn_BandwidthConstrainedComm.

GNN message passing: per batch element, N=256 agents each generate a
message (MLP -> compress -> decompress), compute pairwise bilinear
relevance scores, pick top-K=8 senders (softmax gated), aggregate their
messages, and run a receiver MLP over [obs, agg].

Sharding: pure data parallel over batch B=128 -> 16 per core x 8 cores.

Implementation notes:
  - transposed [feature, agent] layout everywhere; obs transposed once
    on the host (data staging) so there are no on-chip obs transposes.
  - main-path matmuls in float32r (4x faster than fp32 on the PE at
    free-dim >= 256; ~1.5e-4 rel err measured on HW). The message /
    gate-aggregation path runs in bf16: messages are ~1e-3 of the
    output magnitude, so bf16 there is invisible in the result.
  - top-8 via the DVE Max8 instruction over exp'd scores, gates built
    densely with a fused (E >= t)*E scalar_tensor_tensor whose
    accum_out gives the softmax denominator. Gates are applied as
    [N,N] @ [N,MSG] matmuls -- no gather.
  - softmax without max subtraction (scores bounded ~30 -> exp finite).
  - decompress matmul computes messages directly in [sender, msg]
    layout with the bias folded in via an appended ones-row (k=33).
  - batches processed in pairs so most free dims are 512.
"""

import sys

sys.path.insert(0, "/opt/trn_rl_repo")

import numpy as np

# problem dims (hardcoded per contract)
B, N, D = 128, 256, 256
MSG, CD, K = 64, 32, 8
H1, H2 = 128, 256
NCORES = 8
BPC = B // NCORES  # batches per core

_CACHE = {}


def build_program(bpc=BPC, passes=1):
    import concourse.bacc as bacc
    import concourse.mybir as mybir
    import concourse.tile as tile
    from concourse.masks import make_identity
    from contextlib import ExitStack

    dt = mybir.dt
    f32, f32r, bf16 = dt.float32, dt.float32r, dt.bfloat16
    AF = mybir.ActivationFunctionType
    OP = mybir.AluOpType

    assert bpc % 2 == 0
    npairs = bpc // 2

    nc = bacc.Bacc("TRN2", target_bir_lowering=False, debug=False,
                   num_devices=NCORES)

    obsT_d = nc.dram_tensor("obsT", [bpc, D, N], f32, kind="ExternalInput")
    W1_d = nc.dram_tensor("W1", [D, H1], f32, kind="ExternalInput")
    b1_d = nc.dram_tensor("b1", [H1], f32, kind="ExternalInput")
    W2_d = nc.dram_tensor("W2", [H1, MSG], f32, kind="ExternalInput")
    b2_d = nc.dram_tensor("b2", [MSG], f32, kind="ExternalInput")
    Wc_d = nc.dram_tensor("Wc", [MSG, CD], f32, kind="ExternalInput")
    bc_d = nc.dram_tensor("bc", [CD], f32, kind="ExternalInput")
    Wd_d = nc.dram_tensor("Wd", [CD, MSG], f32, kind="ExternalInput")
    bd_d = nc.dram_tensor("bd", [MSG], f32, kind="ExternalInput")
    Wbil_d = nc.dram_tensor("Wbil", [D, D], f32, kind="ExternalInput")
    Wr1_d = nc.dram_tensor("Wr1", [D + MSG, H2], f32, kind="ExternalInput")
    br1_d = nc.dram_tensor("br1", [H2], f32, kind="ExternalInput")
    Wr2_d = nc.dram_tensor("Wr2", [H2, D], f32, kind="ExternalInput")
    br2_d = nc.dram_tensor("br2", [D], f32, kind="ExternalInput")
    out_d = nc.dram_tensor("out", [bpc, N, D], f32, kind="ExternalOutput")

    with tile.TileContext(nc) as tc, ExitStack() as ctx:
        wp = ctx.enter_context(tc.tile_pool(name="wp", bufs=1))
        dp = ctx.enter_context(tc.tile_pool(name="dp", bufs=2))
        sp = ctx.enter_context(tc.tile_pool(name="sp", bufs=2))
        pp = ctx.enter_context(tc.tile_pool(name="pp", bufs=1, space="PSUM"))

        # PSUM banks (8 x 2KB): mlpmix 2, tmp 2, sg 2, rout 2

        # ---------------- one-time setup ----------------
        ident = wp.tile([128, 128], f32)
        make_identity(nc, ident[:])
        ident_b = wp.tile([128, 128], bf16)
        nc.vector.tensor_copy(ident_b[:], ident[:])
        warm_ps = pp.tile([128, 128], f32, tag="mlpmix", bufs=2)
        nc.tensor.transpose(warm_ps[:], ident[:], ident[:])

        def load_cast(dram_ap, shape, name, cdt):
            t_f = wp.tile(shape, f32, name=name + "_f")
            nc.sync.dma_start(t_f[:], dram_ap)
            t_r = wp.tile(shape, cdt, name=name + "_r")
            nc.vector.tensor_copy(t_r[:], t_f[:])
            return t_r

        W1_r0 = load_cast(W1_d[0:128, :], [128, H1], "W1a", bf16)
        W1_r1 = load_cast(W1_d[128:256, :], [128, H1], "W1b", bf16)
        W2_b = load_cast(W2_d[:], [H1, MSG], "W2", bf16)
        Wc_b = load_cast(Wc_d[:], [MSG, CD], "Wc", bf16)
        Wb_r0 = load_cast(Wbil_d[0:128, :], [128, D], "Wba", bf16)
        Wb_r1 = load_cast(Wbil_d[128:256, :], [128, D], "Wbb", bf16)
        Wr1_r0 = load_cast(Wr1_d[0:128, :], [128, H2], "Wr1a", bf16)
        Wr1_r1 = load_cast(Wr1_d[128:256, :], [128, H2], "Wr1b", bf16)
        Wr1_r2 = load_cast(Wr1_d[256:320, :], [MSG, H2], "Wr1c", bf16)
        Wr2_r0 = load_cast(Wr2_d[0:128, :], [128, D], "Wr2a", bf16)
        Wr2_r1 = load_cast(Wr2_d[128:256, :], [128, D], "Wr2b", bf16)

        # Wd with bd folded in as a 33rd contraction row (bf16)
        Wd_f = wp.tile([CD, MSG], f32)
        nc.sync.dma_start(Wd_f[:], Wd_d[:])
        bd_row_f = wp.tile([1, MSG], f32)
        nc.sync.dma_start(bd_row_f[:],
                          bd_d[:].rearrange("(o m) -> o m", o=1))
        Wd_bd_b = wp.tile([CD + 1, MSG], bf16)
        nc.vector.tensor_copy(Wd_bd_b[0:CD, :], Wd_f[:])
        nc.vector.tensor_copy(Wd_bd_b[CD:CD + 1, :], bd_row_f[:])

        def load_bias(dram, p, name, off=0):
            t = wp.tile([p, 1], f32, name=name)
            nc.sync.dma_start(
                t[:], dram[off:off + p].rearrange("(p o) -> p o", o=1))
            return t

        b1_sb = load_bias(b1_d, H1, "b1s")
        b2_sb = load_bias(b2_d, MSG, "b2s")
        bc_sb = load_bias(bc_d, CD, "bcs")
        br1_sb0 = load_bias(br1_d, 128, "br1s0")
        br1_sb1 = load_bias(br1_d, 128, "br1s1", off=128)

        # ones row + br2 row for folding br2 into the last matmul
        ones_f = wp.tile([1, 128], f32)
        nc.vector.memset(ones_f[:], 1.0)
        ones_r = wp.tile([1, 128], bf16)
        nc.vector.tensor_copy(ones_r[:], ones_f[:])
        br2row_f = wp.tile([1, D], f32)
        nc.sync.dma_start(br2row_f[:],
                          br2_d[:].rearrange("(o d) -> o d", o=1))
        br2row_r = wp.tile([1, D], bf16)
        nc.vector.tensor_copy(br2row_r[:], br2row_f[:])

        # persistent double-buffered compressed-message tiles with a
        # constant ones-row (row CD) for the folded bd bias
        cT_tiles = []
        for i in range(2):
            t = wp.tile([CD + 1, 2 * N], bf16, name=f"cTp{i}")
            nc.vector.memset(t[CD:CD + 1, :], 1.0)
            cT_tiles.append(t)

        # ---------------- main loop over batch pairs ----------------
        for _ in range(passes):
            for p in range(npairs):
                b0 = 2 * p
                od_b = []
                for dc in range(2):
                    of = dp.tile([128, 2, N], f32, name=f"od{dc}_f",
                                 tag=f"od{dc}f")
                    nc.sync.dma_start(
                        of[:], obsT_d[b0:b0 + 2, 128 * dc:128 * (dc + 1), :]
                        .rearrange("b d n -> d b n"))
                    ob = dp.tile([128, 2 * N], bf16, name=f"od{dc}_b",
                                 tag=f"od{dc}b")
                    nc.vector.tensor_copy(
                        ob[:], of[:].rearrange("d b n -> d (b n)"))
                    od_b.append(ob)

                # ---- message MLP (pair-wide, bf16 after first layer) ----
                hT_ps = pp.tile([H1, 2 * N], f32, tag="mlpmix", bufs=2)
                nc.tensor.matmul(hT_ps[:], W1_r0[:], od_b[0][:],
                                 start=True, stop=False)
                nc.tensor.matmul(hT_ps[:], W1_r1[:], od_b[1][:],
                                 start=False, stop=True)
                hT_b = sp.tile([H1, 2 * N], bf16, name="hT_b")
                nc.scalar.activation(hT_b[:], hT_ps[:], AF.Relu,
                                     bias=b1_sb[:])

                mgT_ps = pp.tile([MSG, 2 * N], f32, tag="mlpmix", bufs=2)
                nc.tensor.matmul(mgT_ps[:], W2_b[:], hT_b[:],
                                 start=True, stop=True)
                mgT_b = sp.tile([MSG, 2 * N], bf16, name="mgT_b")
                nc.vector.tensor_scalar_add(mgT_b[:], mgT_ps[:], b2_sb[:])

                cT_ps = pp.tile([CD, 2 * N], f32, tag="mlpmix", bufs=2)
                nc.tensor.matmul(cT_ps[:], Wc_b[:], mgT_b[:],
                                 start=True, stop=True)
                cT_b = cT_tiles[p % 2]
                nc.vector.tensor_scalar_add(cT_b[0:CD, :], cT_ps[:],
                                            bc_sb[:])

                # ---- bilinear tmp (pair-wide, fp32r) ----
                tmpT_r = []
                for ec in range(2):
                    tps = pp.tile([128, 2 * N], f32, tag="tmp", bufs=2,
                                  name=f"tmp{ec}_ps")
                    nc.tensor.matmul(tps[:],
                                     Wb_r0[:, 128 * ec:128 * (ec + 1)],
                                     od_b[0][:], start=True, stop=False)
                    nc.tensor.matmul(tps[:],
                                     Wb_r1[:, 128 * ec:128 * (ec + 1)],
                                     od_b[1][:], start=False, stop=True)
                    trr = sp.tile([128, 2 * N], bf16, name=f"tmp{ec}_r",
                                  tag=f"tmp{ec}r")
                    nc.scalar.activation(trr[:], tps[:], AF.Copy)
                    tmpT_r.append(trr)

                aggT_ps = pp.tile([MSG, 2 * N], f32, tag="mlpmix", bufs=2)

                for bi in range(2):
                    boff = bi * N

                    # messages directly in [sender, msg] layout, bias
                    # folded via the ones-row (k = CD+1)
                    msn_ps = pp.tile([128, 2, MSG], f32, tag="mlpmix",
                                     bufs=2, name="msn_ps")
                    for jc in range(2):
                        nc.tensor.matmul(
                            msn_ps[:, jc, :],
                            cT_b[:, boff + 128 * jc:boff + 128 * (jc + 1)],
                            Wd_bd_b[:], start=True, stop=True)
                    msgs_b = sp.tile([128, 2, MSG], bf16, name="msgs_b")
                    nc.vector.tensor_copy(msgs_b[:], msn_ps[:])

                    # scores for this batch: [i_chunk(128), (ic, j)(512)]
                    s_ps = pp.tile([128, 2, N], f32, tag="sg", bufs=2,
                                   name="s_ps")
                    for ic in range(2):
                        ioff = boff + 128 * ic
                        nc.tensor.matmul(s_ps[:, ic, :],
                                         tmpT_r[0][:, ioff:ioff + 128],
                                         od_b[0][:, boff:boff + N],
                                         start=True, stop=False)
                        nc.tensor.matmul(s_ps[:, ic, :],
                                         tmpT_r[1][:, ioff:ioff + 128],
                                         od_b[1][:, boff:boff + N],
                                         start=False, stop=True)

                    # gating (bf16: message path is ~1e-3 of the output,
                    # so bf16 gate errors are invisible)
                    E = sp.tile([128, 2, N], bf16, name="E")
                    nc.scalar.activation(E[:], s_ps[:], AF.Exp)
                    Gt_ps = pp.tile([128, 2, N], bf16, tag="sg", bufs=2,
                                    name="Gt_ps")
                    for ic in range(2):
                        top8 = sp.tile([128, 8], bf16, name="top8")
                        nc.vector.max(out=top8[:], in_=E[:, ic, :])
                        U = sp.tile([128, N], bf16, name="U")
                        den = sp.tile([128, 1], f32, name="den")
                        nc.vector.scalar_tensor_tensor(
                            out=U[:], in0=E[:, ic, :], scalar=top8[:, 7:8],
                            in1=E[:, ic, :], op0=OP.is_ge, op1=OP.mult,
                            accum_out=den[:])
                        rden = sp.tile([128, 1], f32, name="rden")
                        nc.vector.reciprocal(rden[:], den[:])
                        G_b = sp.tile([128, N], bf16, name="G_b")
                        nc.vector.tensor_scalar_mul(G_b[:], U[:],
                                                    rden[:, 0:1])
                        for jc in range(2):
                            nc.tensor.transpose(
                                Gt_ps[:, jc, 128 * ic:128 * (ic + 1)],
                                G_b[:, 128 * jc:128 * (jc + 1)],
                                ident_b[:])

                    Gt_b = sp.tile([128, 2, N], bf16, name="Gt_b")
                    nc.scalar.activation(Gt_b[:], Gt_ps[:], AF.Copy)

                    # aggT[m, i] = sum_j msgs[j, m] * Gt[j, i]
                    nc.tensor.matmul(aggT_ps[:, boff:boff + N],
                                     msgs_b[:, 0, :], Gt_b[:, 0, :],
                                     start=True, stop=False)
                    nc.tensor.matmul(aggT_ps[:, boff:boff + N],
                                     msgs_b[:, 1, :], Gt_b[:, 1, :],
                                     start=False, stop=True)

                aggT_r = sp.tile([MSG, 2 * N], bf16, name="aggT_r")
                nc.vector.tensor_copy(aggT_r[:], aggT_ps[:])

                # ---- receiver MLP (pair-wide, fp32r) ----
                rT_r = []
                for mi in range(2):
                    rps = pp.tile([128, 2 * N], f32, tag="rout", bufs=2,
                                  name=f"r{mi}_ps")
                    ms = 128 * mi
                    nc.tensor.matmul(rps[:], Wr1_r0[:, ms:ms + 128],
                                     od_b[0][:], start=True, stop=False)
                    nc.tensor.matmul(rps[:], Wr1_r1[:, ms:ms + 128],
                                     od_b[1][:], start=False, stop=False)
                    nc.tensor.matmul(rps[:], Wr1_r2[:, ms:ms + 128],
                                     aggT_r[:], start=False, stop=True)
                    rr = sp.tile([128, 2 * N], bf16, name=f"r{mi}_r",
                                 tag=f"r{mi}r")
                    nc.scalar.activation(
                        rr[:], rps[:], AF.Relu,
                        bias=(br1_sb0 if mi == 0 else br1_sb1)[:])
                    rT_r.append(rr)

                for bi in range(2):
                    b = b0 + bi
                    boff = bi * N
                    out_ps = pp.tile([128, 2, D], f32, tag="mlpmix",
                                     bufs=2, name="out_ps")
                    for ni in range(2):
                        ls = boff + 128 * ni
                        nc.tensor.matmul(out_ps[:, ni, :],
                                         rT_r[0][:, ls:ls + 128],
                                         Wr2_r0[:], start=True, stop=False)
                        nc.tensor.matmul(out_ps[:, ni, :],
                                         rT_r[1][:, ls:ls + 128],
                                         Wr2_r1[:], start=False, stop=False)
                        nc.tensor.matmul(out_ps[:, ni, :],
                                         ones_r[:, 0:128], br2row_r[:],
                                         start=False, stop=True)
                    out_sb = sp.tile([128, 2, D], f32, name="out_sb")
                    nc.scalar.activation(out_sb[:], out_ps[:], AF.Copy)
                    nc.sync.dma_start(
                        out_d[b].rearrange("(c p) d -> p c d", p=128),
                        out_sb[:])

    nc.compile()
    return nc


def _np_inputs_for_core(inputs, core):
    obs = inputs["obs_all"]
    lo = core * BPC
    obsT = np.ascontiguousarray(
        obs[lo:lo + BPC].transpose(0, 2, 1)).astype(np.float32)
    m = {"obsT": obsT}
    for k in ("W1", "b1", "W2", "b2", "Wc", "bc", "Wd", "bd", "Wbil",
              "Wr1", "br1", "Wr2", "br2"):
        m[k] = np.ascontiguousarray(inputs[k]).astype(np.float32)
    return m


def kernel(**inputs):
    from concourse.bass_utils import run_bass_kernel_spmd

    if "prog" not in _CACHE:
        _CACHE["prog"] = build_program(BPC)
    nc = _CACHE["prog"]

    core_ids = list(range(NCORES))
    in_maps = [_np_inputs_for_core(inputs, c) for c in core_ids]
    res = run_bass_kernel_spmd(nc, in_maps, core_ids)
    out = np.concatenate([res.results[c]["out"] for c in core_ids], axis=0)
    return out.astype(np.float32)
